# revision 28
# baseline (speedup 1.0000x reference)
"""Trainium2 Bass kernel: causal attention with weight-normed QKV projections.

Problem (hardcoded): B=8, Cq=Ck=256, C=512, H=W=32 -> S=1024, N_HEAD=8, dh=64.
Sharding: pure data-parallel over batch (8 batches -> 8 cores), weights
replicated. No collectives.

Host-side glue folds the weight-norm scale g/||v_row|| into the weights and
pre-transposes them (w^T = [CIN, C] bf16), so the device does no weight prep
at all: wt = [wq^T; wk^T; wv^T] bf16, qk = [q;k] bf16, gb = [bq;bk] fp32,
bv fp32, msk = strict-upper-ones bf16.

Per-core pipeline (batch b):
  1. PE warm-up: a burst of dummy matmuls on a zeroed tile while the input
     DMAs land, so the HAM clock gate reaches 8/8 before real work starts.
  2. Four head-pair phases (a2 = ct = 0..3). Each phase runs its own QK ->
     exp -> PV chain ("same-phase PV", one j-group delayed so the PE never
     sits behind ACT's exp), with the NEXT phase's projections sprinkled
     between j-groups. V projections (bias added via DVE, ones column per
     head -> PV also yields softmax denominators) are sprinkled through
     phase 0. Output tiles complete in ascending order and are DMA'd out
     immediately, so the store overlaps compute.
  3. Attention per head pair: K=64 QK matmuls row-group packed (the two
     heads live at partitions 0-63 / 64-127 of their C-tile), per-head
     [128, nj] logit psum tiles, exp per head straight out of PSUM into a
     shared bf16 e-tile. Strictly-causal: only the lower triangle of
     [128,128] tiles is computed; diagonal tiles of a whole j-group x both
     heads get one strided-AP strict-upper mask multiply after exp. No max
     subtraction: logits are O(10).
     PV: out[SqTile, 65] accumulated over S_k tiles with e^T slices as the
     stationary operand; both heads' rows are normalized by one strided-AP
     multiply with the reciprocal of column 64.
  4. Output stored [S, C] fp32; host transposes to [C, H, W].
"""

import numpy as np

import concourse.bass as bass
import concourse.tile as tile
from concourse import bacc, mybir
from concourse.bass_utils import run_bass_kernel_spmd

F32 = mybir.dt.float32
BF16 = mybir.dt.bfloat16
AF = mybir.ActivationFunctionType
ALU = mybir.AluOpType

S = 1024          # sequence length (32*32)
CIN = 256         # input channels (Cq = Ck)
C = 512           # projection channels
NH = 8            # heads
DH = 64           # head dim
HW = 32           # spatial H = W
N_CORES = 8

# causal work list: the lower-triangle row-strips split into <=512-column
# q-chunks (one psum bank per head), ordered so the early q-tiles' PV chains
# unlock first.  Each entry is a logit tile: a list of (strip j, q0, width).
# Chunks are aligned to the 512-column boundary so chunks 0-3 need only the
# n=0 projection halves: the exp stream starts after ~1MB of prioritized DMA
# and covers the remaining input transfers.
CHUNKS = [
    [(0, 0, 512)], [(1, 128, 384)], [(2, 256, 256), (3, 384, 128)],
    [(0, 512, 512)], [(1, 512, 512)], [(2, 512, 512)], [(3, 512, 512)],
    [(4, 512, 512)], [(5, 640, 384)], [(6, 768, 256), (7, 896, 128)],
]
# phase 3 splits the tail tile so PV(3,6) overlaps strip 7's exp
CHUNKS3 = CHUNKS[:9] + [[(6, 768, 256)], [(7, 896, 128)]]
# chunk -> PV chains to emit there (deferred past readiness so nothing can
# FIFO-block the QK stream); the big chains [6, 7] carry into the next
# phase's first two chunks
PV_AT = {4: [0], 5: [1], 6: [2], 7: [3], 8: [4], 9: [5]}
# chunk -> V projections to emit there (phase 0 only; wtv is the last DMA)
V_AT = {3: [0, 1], 4: [2, 3], 7: [4], 8: [5], 9: [6, 7]}
# off-diagonal chunks whose exp runs as a one-instruction DVE Schraudolph
# fast-exp instead of on ACT (per-phase chunk indices; all segs mask-free
# there).
SCHRAUD_AT = {3, 4, 6}
# bf16-level Schraudolph: int16(l * C0 + C1) IS the bf16 bit pattern of
# ~exp(l/8) (exponent in the top byte, 7-bit linear mantissa).  Balanced
# +-2.1% interp error; the softmax ratio cancels most of it.  The int16
# result is written straight into the e-tile viewed as int16 - one DVE op.
_SCH_C0 = float((1 << 7) / np.log(2.0) * 0.125)
_SCH_C1 = float(127 * (1 << 7) + np.log2(1.0212) * (1 << 7) + 0.5)


def _build_module():
    nc = bacc.Bacc("TRN2", target_bir_lowering=False)

    wt_d = nc.dram_tensor("wt", [6 * 128, C], BF16, kind="ExternalInput").ap()
    qk_d = nc.dram_tensor("qk", [2 * CIN, S], BF16, kind="ExternalInput").ap()
    gb_d = nc.dram_tensor("gb", [2, C], F32, kind="ExternalInput").ap()
    bv_d = nc.dram_tensor("bv", [C], F32, kind="ExternalInput").ap()
    msk_d = nc.dram_tensor("msk", [128, 128], BF16, kind="ExternalInput").ap()
    o_d = nc.dram_tensor("o", [S, C], F32, kind="ExternalOutput").ap()

    with tile.TileContext(nc) as tc:
        with (
            tc.tile_pool(name="const", bufs=1) as const,
            tc.tile_pool(name="persist", bufs=1) as persist,
            tc.tile_pool(name="smalls", bufs=4) as smalls,
        ):
            # ---- warm-up scratch (no DMA dependency at all)
            wrm = const.tile([128, 512], BF16, name="wrm")
            nc.gpsimd.memset(wrm, 0.0)
            # ---- input DMAs, spread across three rings; earliest-needed
            # first.  The first projection (ct=0, S 0:512) only needs the
            # ct=0 weight columns and the S 0:512 input halves, so those are
            # split into small pieces that land within ~2us of ring start.
            wt_sb = persist.tile([128, 6, C], BF16, name="wt_sb")
            def _wt_chunk(ring, n, c0=0, c1=C):
                ring.dma_start(
                    out=wt_sb[:, n:n + 1, c0:c1],
                    in_=wt_d[128 * n:128 * (n + 1), c0:c1].rearrange(
                        "(n p) c -> p n c", p=128))
            qhalves = [persist.tile([128, 2, S], BF16, name=f"qk{half}")
                       for half in range(2)]
            def _qk_piece(ring, half, kc, c0, c1):
                ring.dma_start(
                    out=qhalves[half][:, kc:kc + 1, c0:c1],
                    in_=qk_d[256 * half + 128 * kc:256 * half + 128 * kc + 128,
                             c0:c1].rearrange("(n p) i -> p n i", p=128))
            def _qk_chunk(ring, half, cc):
                ring.dma_start(
                    out=qhalves[half][:, :, 512 * cc:512 * (cc + 1)],
                    in_=qk_d[256 * half:256 * (half + 1),
                             512 * cc:512 * (cc + 1)].rearrange(
                        "(n p) i -> p n i", p=128))
            # tiny bias/mask constants first (they gate the first epilogue
            # and the first diagonal mask), then q weights + q data + k
            # weights land in parallel on the three rings.  The ACT (scalar)
            # sequencer is NEVER used for DMA triggers: each trigger costs
            # ~667ns of sequencer time and delays the exp stream.
            gb_sb = const.tile([128, 8], F32, name="gb_sb")
            triu = const.tile([128, 128], BF16, name="triu")
            # Input DMAs ride the SP + GPSIMD rings only (an ACT-ring trigger
            # costs ~667ns of the exp engine's sequencer); a few late pieces
            # are emitted mid-stream below so their ACT triggers land between
            # exps, after the critical head.  Order is first-need: Q proj
            # (ct0 weight cols + xq S0:512), K proj, then the rest.
            _qk_piece(nc.gpsimd, 0, 0, 0, 512)   # xq kc0
            _qk_piece(nc.sync, 0, 1, 0, 512)     # xq kc1
            _wt_chunk(nc.gpsimd, 0, 0, 128)      # wq kc0 ct0
            _wt_chunk(nc.sync, 1, 0, 128)        # wq kc1 ct0
            nc.gpsimd.dma_start(out=gb_sb,
                                in_=gb_d.rearrange("n (c p) -> p (n c)", p=128))
            _qk_piece(nc.sync, 1, 0, 0, 512)     # xk kc0
            _qk_piece(nc.gpsimd, 1, 1, 0, 512)   # xk kc1
            _wt_chunk(nc.sync, 3, 0, 128)        # wk kc1 ct0
            _wt_chunk(nc.gpsimd, 2, 0, 128)      # wk kc0 ct0
            nc.sync.dma_start(out=triu, in_=msk_d)
            _wt_chunk(nc.sync, 1, 128, 512)      # Q ct1-3 weights
            _wt_chunk(nc.gpsimd, 0, 128, 512)
            _wt_chunk(nc.sync, 5)                # V weights
            _wt_chunk(nc.gpsimd, 4)
            _qk_piece(nc.sync, 0, 1, 512, 1024)  # xq S 512:1024
            _qk_piece(nc.gpsimd, 0, 0, 512, 1024)
            _qk_piece(nc.sync, 1, 1, 512, 1024)  # xk S 512:1024
            _qk_piece(nc.gpsimd, 1, 0, 512, 1024)
            wT = [[wt_sb[:, 2 * t + kc, :] for kc in range(2)] for t in range(3)]
            qT = [qhalves[0][:, 0, :], qhalves[0][:, 1, :]]
            kTt = [qhalves[1][:, 0, :], qhalves[1][:, 1, :]]
            bq_sb = gb_sb[:, 0:4]
            bk_sb = gb_sb[:, 4:8]
            bvb = const.tile([128, C], F32, name="bvb")

            def _late_dmas(c):
                # late inputs on the ACT ring: their ~667ns triggers execute
                # between the first exps, clear of the critical head.
                if c == 0:
                    nc.scalar.dma_start(
                        out=bvb,
                        in_=bass.AP(tensor=bv_d.tensor, offset=bv_d.offset,
                                    ap=[[0, 128]] + list(bv_d.ap)),
                    )
                elif c == 1:
                    _wt_chunk(nc.scalar, 2, 128, 512)   # K ct1-3 weights
                    _wt_chunk(nc.scalar, 3, 128, 512)

            QT = [persist.tile([128, S], BF16, name=f"QT{ct}") for ct in range(4)]
            KT = [persist.tile([128, S], BF16, name=f"KT{ct}") for ct in range(4)]
            VP = [persist.tile([128, NH * 65], BF16, name=f"VP{st}")
                  for st in range(8)]
            OUT = [persist.tile([128, C], F32, name=f"OUT{i}") for i in range(8)]

            with (
                tc.tile_pool(name="psW", bufs=1, space="PSUM") as psW,
                tc.tile_pool(name="psL", bufs=1, space="PSUM") as psL,
                tc.tile_pool(name="psPV", bufs=2, space="PSUM") as psPV,
                tc.tile_pool(name="epool", bufs=2) as epool,
            ):
                # ---- HAM warm-up: dummy matmuls bridge the input-DMA wait so
                # the PE clock gate reaches 8/8 by the first real matmul.
                for i in range(12):
                    wp = psW.tile([128, 512], F32, tag="pp", bufs=2,
                                  name=f"warm{i}")
                    nc.tensor.matmul(wp[:, 0:256], lhsT=wrm[:, 0:128],
                                     rhs=wrm[:, 0:256], start=True, stop=True)
                gchunk = [0]   # global chunk counter -> psL ping-pong parity

                def emit_proj_group(ct, g):
                    # g in 0..3 -> (q/k, n-half); scale pre-folded on host
                    dst, wpair, src, b_sb, pnm = (
                        (QT, wT[0], qT, bq_sb, "q"),
                        (KT, wT[1], kTt, bk_sb, "k"),
                    )[g // 2]
                    n = g % 2
                    pp = psW.tile([128, 512], F32, tag="pp", bufs=2,
                                  name=f"pp{pnm}{ct}_{n}")
                    for kc in range(2):
                        nc.tensor.matmul(
                            pp,
                            lhsT=wpair[kc][:, 128 * ct:128 * (ct + 1)],
                            rhs=src[kc][:, 512 * n:512 * (n + 1)],
                            start=(kc == 0), stop=(kc == 1),
                        )
                    # K carries no bias: logits (q+bq).(k+bk) differ from
                    # (q+bq).k only by a per-query-row constant q.bk, which
                    # cancels exactly in the softmax.  So K evicts as a plain
                    # Copy on the ACT engine (Copy is not table-based, so it
                    # never evicts the Exp table), offloading the DVE.
                    is_q = g // 2 == 0
                    if ct == 0 and g == 2:
                        # first QK matmul only needs KT[0][:, 0:128]
                        for c0, c1 in ((0, 128), (128, 512)):
                            nc.scalar.copy(dst[ct][:, c0:c1], pp[:, c0:c1])
                    elif is_q:
                        nc.vector.tensor_scalar_add(
                            out=dst[ct][:, 512 * n:512 * (n + 1)],
                            in0=pp,
                            scalar1=b_sb[:, ct:ct + 1],
                        )
                    else:
                        nc.scalar.copy(
                            dst[ct][:, 512 * n:512 * (n + 1)], pp)

                def emit_v(st):
                    vp = VP[st]
                    ppv = psW.tile([128, 512], F32, tag="pp", bufs=2,
                                   name=f"ppv{st}")
                    for kc in range(2):
                        nc.tensor.matmul(
                            ppv,
                            lhsT=kTt[kc][:, 128 * st:128 * (st + 1)],
                            rhs=wT[2][kc],
                            start=(kc == 0), stop=(kc == 1),
                        )
                    vp3 = vp.rearrange("p (h c) -> p h c", c=65)
                    nc.gpsimd.memset(vp3[:, :, 64:65], 1.0)
                    nc.vector.tensor_add(
                        vp3[:, :, 0:64],
                        ppv.rearrange("p (h c) -> p h c", c=64),
                        bvb.rearrange("p (h c) -> p h c", c=64),
                    )

                def emit_L(a2, c, segs, esegs):
                    W = sum(w for _, _, w in segs)
                    e = epool.tile([128, 2 * W], BF16, tag=f"e_{c}",
                                   name=f"e_{a2}_{c}")
                    off = 0
                    for (j, q0, w) in segs:
                        esegs.setdefault(j, []).append((e, W, off, q0, w))
                        off += w
                    # full [128, 2, 512] so the head planes stay bank-aligned;
                    # parity from a global counter so phase boundaries keep
                    # ping-ponging instead of colliding on the same buffer
                    gchunk[0] += 1
                    lt = psL.tile([128, 2, 512], F32, tag=f"lt{gchunk[0] % 2}",
                                  name=f"lt_{a2}_{c}")
                    for hi in range(2):
                        p0 = 64 * hi
                        off = 0
                        for (j, q0, w) in segs:
                            nc.tensor.matmul(
                                lt[:, hi, off:off + w],
                                lhsT=KT[a2][p0:p0 + 64, 128 * j:128 * j + 128],
                                rhs=QT[a2][p0:p0 + 64, q0:q0 + w],
                                start=True, stop=True,
                            )
                            off += w
                    if c in SCHRAUD_AT:
                        # one-op DVE Schraudolph fast-exp: int16(l*C0 + C1)
                        # written into the e tile viewed as int16 IS bf16
                        # ~exp(l/8).  Frees ACT time at no extra passes.
                        nc.vector.tensor_scalar(
                            out=e.rearrange("p (h c) -> p h c",
                                            h=2).bitcast(mybir.dt.int16),
                            in0=lt[:, :, 0:W],
                            scalar1=_SCH_C0, scalar2=_SCH_C1,
                            op0=ALU.mult, op1=ALU.add)
                    else:
                        # one exp covering both heads
                        nc.scalar.activation(
                            out=e.rearrange("p (h c) -> p h c", h=2),
                            in_=lt[:, :, 0:W], func=AF.Exp, scale=0.125)
                    # one strided-AP strict-upper mask multiply covering the
                    # diagonal tiles of both heads (segs starting on-diagonal)
                    dsegs = [off_ for (j, q0, w), off_ in zip(
                        segs, np.cumsum([0] + [w for _, _, w in segs[:-1]]))
                        if q0 == 128 * j]
                    if dsegs:
                        dims = [list(e.ap[0]), [W, 2]]
                        tdims = [list(triu.ap[0]), [0, 2]]
                        if len(dsegs) == 2:
                            dims.append([dsegs[1] - dsegs[0], 2])
                            tdims.append([0, 2])
                        dims.append([1, 128])
                        tdims.append([1, 128])
                        ev = bass.AP(tensor=e.tensor,
                                     offset=e.offset + int(dsegs[0]), ap=dims)
                        tv = bass.AP(tensor=triu.tensor, offset=triu.offset,
                                     ap=tdims)
                        # diagonal masks run on the otherwise-idle GPSIMD
                        nc.gpsimd.tensor_mul(ev, ev, tv)

                def _pv_acc(po, a2, i, esegs, jj0, jj1, start, stop):
                    # accumulate strips jj0..jj1 of PV(i) for the head pair;
                    # start/stop open/close the psum accumulation group so a
                    # chain can be split across chunks (deferred tail terms).
                    # PSUM zero-regions are bank-sized: exactly ONE start=True
                    # is issued per bank (the lazy zero covers the second
                    # head's bytes on their first write) so a later start can
                    # never re-mark already-accumulated bytes pending-zero.
                    for hi in range(2):
                        hh = 2 * a2 + hi
                        for jj in range(jj0, jj1 + 1):
                            for (et, W, off, q0, w) in esegs[jj]:
                                if q0 <= 128 * i < q0 + w:
                                    base = hi * W + off + 128 * i - q0
                                    break
                            nc.tensor.matmul(
                                po[:, 65 * hi:65 * hi + 65],
                                lhsT=et[:, base:base + 128],
                                rhs=VP[jj][:, 65 * hh:65 * hh + 65],
                                start=(start and hi == 0 and jj == jj0),
                                stop=(stop and hi == 1 and jj == jj1),
                                skip_group_check=True,
                            )

                def norm_po(a2, i, po):
                    r = smalls.tile([128, 2], F32, tag="r", name=f"r{a2}_{i}")
                    nc.vector.reciprocal_approx_fast(
                        r, po.rearrange("p (g x) -> p g x", g=2)[:, :, 64:65])
                    # one strided-AP normalize covering both heads
                    r_v = bass.AP(tensor=r.tensor, offset=r.offset,
                                  ap=[list(r.ap[0]), [1, 2], [0, 64]])
                    nc.vector.tensor_mul(
                        OUT[i][:, 128 * a2:128 * (a2 + 1)].rearrange(
                            "p (h c) -> p h c", c=64),
                        po.rearrange("p (h c) -> p h c", c=65)[:, :, 0:64],
                        r_v,
                    )

                def emit_PV(a2, i, esegs):
                    # both heads accumulate into one 1-bank psum tile
                    po = psPV.tile([128, 130], F32, tag="po",
                                   name=f"po_{a2}_{i}")
                    _pv_acc(po, a2, i, esegs, 0, i, start=True, stop=True)
                    norm_po(a2, i, po)

                def emit_PV_part(a2, i, esegs, jj0, jj1, tag):
                    # a complete closed accumulation group over jj0..jj1
                    pool = psW if tag == "pp" else psPV
                    po = pool.tile([128, 130], F32, tag=tag, bufs=2,
                                   name=f"pop_{a2}_{i}_{jj0}")
                    for hi in range(2):
                        hh = 2 * a2 + hi
                        for jj in range(jj0, jj1 + 1):
                            for (et, W, off, q0, w) in esegs[jj]:
                                if q0 <= 128 * i < q0 + w:
                                    base = hi * W + off + 128 * i - q0
                                    break
                            nc.tensor.matmul(
                                po[:, 65 * hi:65 * hi + 65],
                                lhsT=et[:, base:base + 128],
                                rhs=VP[jj][:, 65 * hh:65 * hh + 65],
                                start=(jj == jj0), stop=(jj == jj1),
                            )
                    return po

                def finish_tail(i, pa_sbuf, pb):
                    # merge the two partials on DVE, then normalize
                    m = smalls.tile([128, 130], F32, tag="m", name=f"m{i}")
                    nc.vector.tensor_add(m, pa_sbuf, pb)
                    r = smalls.tile([128, 2], F32, tag="r", name=f"rt{i}")
                    nc.vector.reciprocal(
                        r, m.rearrange("p (g x) -> p g x", g=2)[:, :, 64:65])
                    r_v = bass.AP(tensor=r.tensor, offset=r.offset,
                                  ap=[list(r.ap[0]), [1, 2], [0, 64]])
                    nc.vector.tensor_mul(
                        OUT[i][:, 384:512].rearrange("p (h c) -> p h c", c=64),
                        m.rearrange("p (h c) -> p h c", c=65)[:, :, 0:64],
                        r_v,
                    )

                # outputs all ride the SP HWDGE ring: SP's sequencer is idle
                # and HWDGE avoids the Q7 SWDGE drain that stretched the tail
                out_rings = [nc.sync, nc.sync]

                def post_pv(a2, i):
                    # ship each half of OUT[i] as soon as its last head-pair
                    # phase has normalized into it (query row 0 attends to
                    # nothing: reference zeroes it first)
                    if a2 == 1:
                        if i == 0:
                            nc.vector.memset(OUT[0][0:1, 0:256], 0.0)
                        out_rings[i % 2].dma_start(
                            out=o_d[128 * i:128 * (i + 1), 0:256],
                            in_=OUT[i][:, 0:256])
                    elif a2 == 3:
                        if i == 0:
                            nc.vector.memset(OUT[0][0:1, 256:512], 0.0)
                        out_rings[i % 2].dma_start(
                            out=o_d[128 * i:128 * (i + 1), 256:512],
                            in_=OUT[i][:, 256:512])

                emit_proj_group(0, 0)   # Q-n0
                emit_proj_group(0, 2)   # K-n0: chunks 0-3 need nothing else
                PROJ_AT = {2: 0, 4: 1, 5: 2, 7: 3}
                carry = []
                po6 = po7 = None
                for a2 in range(4):
                    chunks = CHUNKS if a2 < 3 else CHUNKS3
                    esegs = {}
                    for c in range(len(chunks)):
                        if a2 == 0 and c == 3:
                            emit_proj_group(0, 1)   # Q-n1 (data lands late)
                        if a2 == 0 and c == 5:
                            emit_proj_group(0, 3)   # K-n1
                        if a2 < 3 and c in PROJ_AT:
                            emit_proj_group(a2 + 1, PROJ_AT[c])
                        emit_L(a2, c, chunks[c], esegs)
                        if a2 == 0:
                            _late_dmas(c)
                            for i in V_AT.get(c, []):
                                emit_v(i)
                        if c < len(carry):
                            pa, pi, pes = carry[c]
                            emit_PV(pa, pi, pes)
                            post_pv(pa, pi)
                        # last phase: split the two tail PV chains so only
                        # their final diagonal terms trail the last exps.
                        # Deferred closes are emitted BEFORE any new psPV
                        # allocation at the same chunk so pool buffer reuse
                        # never overtakes an open chain.
                        if a2 == 3 and c == 9:
                            _pv_acc(po6, 3, 6, esegs, 6, 6,
                                    start=False, stop=True)
                            norm_po(3, 6, po6)
                            post_pv(3, 6)
                            _pv_acc(po7, 3, 7, esegs, 6, 6,
                                    start=False, stop=False)
                        for i in PV_AT.get(c, []):
                            emit_PV(a2, i, esegs)
                            post_pv(a2, i)
                        if a2 == 3 and c == 8:
                            po6 = psPV.tile([128, 130], F32, tag="po",
                                            name="po_d6")
                            _pv_acc(po6, 3, 6, esegs, 0, 5,
                                    start=True, stop=False)
                            po7 = psPV.tile([128, 130], F32, tag="po",
                                            name="po_d7")
                            _pv_acc(po7, 3, 7, esegs, 0, 5,
                                    start=True, stop=False)
                        if a2 == 3 and c == 10:
                            _pv_acc(po7, 3, 7, esegs, 7, 7,
                                    start=False, stop=True)
                            norm_po(3, 7, po7)
                            post_pv(3, 7)
                    carry = [(a2, 6, esegs), (a2, 7, esegs)]
    nc.compile()
    return nc


_CACHE = {}


def _get_module():
    if "nc" not in _CACHE:
        _CACHE["nc"] = _build_module()
    return _CACHE["nc"]


def _in_maps(inputs):
    import ml_dtypes

    q = np.asarray(inputs["query"], dtype=np.float32)
    k = np.asarray(inputs["key"], dtype=np.float32)
    B = q.shape[0]
    assert B == N_CORES
    # fold weight-norm scale g/||v_row|| into the weights; pre-transpose
    wts = []
    for nm in ("q", "k", "v"):
        v = np.asarray(inputs[f"v{nm}"], np.float64)
        g = np.asarray(inputs[f"g{nm}"], np.float64)
        w = g[:, None] * v / np.linalg.norm(v, axis=1, keepdims=True)
        wts.append(w.T)                               # [CIN, C]
    wt = np.ascontiguousarray(
        np.concatenate(wts, axis=0).astype(ml_dtypes.bfloat16))
    gb = np.ascontiguousarray(np.stack(
        [np.asarray(inputs["bq"], np.float32),
         np.asarray(inputs["bk"], np.float32)]))
    bv = np.ascontiguousarray(np.asarray(inputs["bv"], np.float32))
    msk = np.ascontiguousarray(
        np.triu(np.ones((128, 128), np.float32), k=1).astype(ml_dtypes.bfloat16))
    shared = {"wt": wt, "gb": gb, "bv": bv, "msk": msk}
    maps = []
    for b in range(B):
        m = dict(shared)
        m["qk"] = np.ascontiguousarray(np.concatenate(
            [q[b].reshape(CIN, S), k[b].reshape(CIN, S)], axis=0
        ).astype(ml_dtypes.bfloat16))
        maps.append(m)
    return maps


def _gather(results):
    outs = []
    for b in range(N_CORES):
        o = np.asarray(results[b]["o"], np.float32)   # [S, C]
        outs.append(np.ascontiguousarray(o.T).reshape(C, HW, HW))
    return np.stack(outs).astype(np.float32)      # [B, C, H, W]


def run(inputs, **kw):
    """Run on hardware; returns (full_output, BassKernelResults)."""
    nc = _get_module()
    res = run_bass_kernel_spmd(nc, _in_maps(inputs), list(range(N_CORES)), **kw)
    return _gather(res.results), res


def kernel(**inputs):
    out, _ = run(inputs)
    return out



# revision 35
# speedup vs baseline: 1.0781x; 1.0781x over previous
"""Trainium2 Bass kernel: causal attention with weight-normed QKV projections.

Problem (hardcoded): B=8, Cq=Ck=256, C=512, H=W=32 -> S=1024, N_HEAD=8, dh=64.
Sharding: pure data-parallel over batch (8 batches -> 8 cores), weights
replicated. No collectives.

Host-side glue folds the weight-norm scale g/||v_row|| into the weights and
pre-transposes them (w^T = [CIN, C] bf16), so the device does no weight prep
at all: wt = [wq^T; wk^T; wv^T] bf16, qk = [q;k] bf16, gb = [bq;bk] fp32,
bv fp32, msk = strict-upper-ones bf16.

Per-core pipeline (batch b):
  1. PE warm-up: a burst of dummy matmuls on a zeroed tile while the input
     DMAs land, so the HAM clock gate reaches 8/8 before real work starts.
  2. Four head-pair phases (a2 = ct = 0..3). Each phase runs its own QK ->
     exp -> PV chain ("same-phase PV", one j-group delayed so the PE never
     sits behind ACT's exp), with the NEXT phase's projections sprinkled
     between j-groups. V projections (bias added via DVE, ones column per
     head -> PV also yields softmax denominators) are sprinkled through
     phase 0. Output tiles complete in ascending order and are DMA'd out
     immediately, so the store overlaps compute.
  3. Attention per head pair: K=64 QK matmuls row-group packed (the two
     heads live at partitions 0-63 / 64-127 of their C-tile), per-head
     [128, nj] logit psum tiles, exp per head straight out of PSUM into a
     shared bf16 e-tile. Strictly-causal: only the lower triangle of
     [128,128] tiles is computed; diagonal tiles of a whole j-group x both
     heads get one strided-AP strict-upper mask multiply after exp. No max
     subtraction: logits are O(10).
     PV: out[SqTile, 65] accumulated over S_k tiles with e^T slices as the
     stationary operand; both heads' rows are normalized by one strided-AP
     multiply with the reciprocal of column 64.
  4. Output stored [S, C] fp32; host transposes to [C, H, W].
"""

import numpy as np

import concourse.bass as bass
import concourse.tile as tile
from concourse import bacc, mybir
from concourse.bass_utils import run_bass_kernel_spmd

F32 = mybir.dt.float32
BF16 = mybir.dt.bfloat16
AF = mybir.ActivationFunctionType
ALU = mybir.AluOpType

S = 1024          # sequence length (32*32)
CIN = 256         # input channels (Cq = Ck)
C = 512           # projection channels
NH = 8            # heads
DH = 64           # head dim
HW = 32           # spatial H = W
N_CORES = 8

# causal work list: the lower-triangle row-strips split into <=512-column
# q-chunks (one psum bank per head), ordered so the early q-tiles' PV chains
# unlock first.  Each entry is a logit tile: a list of (strip j, q0, width).
# Chunks are aligned to the 512-column boundary so chunks 0-3 need only the
# n=0 projection halves: the exp stream starts after ~1MB of prioritized DMA
# and covers the remaining input transfers.
CHUNKS = [
    [(0, 0, 512)], [(1, 128, 384)], [(2, 256, 256), (3, 384, 128)],
    [(0, 512, 512)], [(1, 512, 512)], [(2, 512, 512)], [(3, 512, 512)],
    [(4, 512, 512)], [(5, 640, 384)], [(6, 768, 256), (7, 896, 128)],
]
# phase 3 splits the tail tile so PV(3,6) overlaps strip 7's exp
CHUNKS3 = CHUNKS[:9] + [[(6, 768, 256)], [(7, 896, 128)]]
# chunk -> PV chains to emit there (deferred past readiness so nothing can
# FIFO-block the QK stream); the big chains [6, 7] carry into the next
# phase's first two chunks
PV_AT = {4: [0], 5: [1], 6: [2], 7: [3], 8: [4], 9: [5]}
# chunk -> V projections to emit there (phase 0 only; wtv is the last DMA)
V_AT = {3: [0, 1], 4: [2, 3], 7: [4], 8: [5], 9: [6, 7]}
# off-diagonal chunks whose exp runs as a one-instruction DVE Schraudolph
# fast-exp instead of on ACT (per-phase chunk indices; all segs mask-free
# there).
SCHRAUD_AT = {4, 6}
# bf16-level Schraudolph: int16(l * C0 + C1) IS the bf16 bit pattern of
# ~exp(l/8) (exponent in the top byte, 7-bit linear mantissa).  Balanced
# +-2.1% interp error; the softmax ratio cancels most of it.  The int16
# result is written straight into the e-tile viewed as int16 - one DVE op.
_SCH_C0 = float((1 << 7) / np.log(2.0) * 0.125)
_SCH_C1 = float(127 * (1 << 7) + np.log2(1.0212) * (1 << 7) + 0.5)


def _build_module():
    nc = bacc.Bacc("TRN2", target_bir_lowering=False)

    wt_d = nc.dram_tensor("wt", [6 * 128, C], BF16, kind="ExternalInput").ap()
    qk_d = nc.dram_tensor("qk", [2 * CIN, S], BF16, kind="ExternalInput").ap()
    gb_d = nc.dram_tensor("gb", [2, C], F32, kind="ExternalInput").ap()
    bv_d = nc.dram_tensor("bv", [C], F32, kind="ExternalInput").ap()
    msk_d = nc.dram_tensor("msk", [128, 128], BF16, kind="ExternalInput").ap()
    o_d = nc.dram_tensor("o", [S, C], F32, kind="ExternalOutput").ap()

    with tile.TileContext(nc) as tc:
        with (
            tc.tile_pool(name="const", bufs=1) as const,
            tc.tile_pool(name="persist", bufs=1) as persist,
            tc.tile_pool(name="smalls", bufs=4) as smalls,
        ):
            # ---- warm-up scratch (no DMA dependency at all)
            wrm = const.tile([128, 512], BF16, name="wrm")
            nc.gpsimd.memset(wrm, 0.0)
            # ---- input DMAs, spread across three rings; earliest-needed
            # first.  The first projection (ct=0, S 0:512) only needs the
            # ct=0 weight columns and the S 0:512 input halves, so those are
            # split into small pieces that land within ~2us of ring start.
            wt_sb = persist.tile([128, 6, C], BF16, name="wt_sb")
            def _wt_chunk(ring, n, c0=0, c1=C):
                ring.dma_start(
                    out=wt_sb[:, n:n + 1, c0:c1],
                    in_=wt_d[128 * n:128 * (n + 1), c0:c1].rearrange(
                        "(n p) c -> p n c", p=128))
            qhalves = [persist.tile([128, 2, S], BF16, name=f"qk{half}")
                       for half in range(2)]
            def _qk_piece(ring, half, kc, c0, c1):
                ring.dma_start(
                    out=qhalves[half][:, kc:kc + 1, c0:c1],
                    in_=qk_d[256 * half + 128 * kc:256 * half + 128 * kc + 128,
                             c0:c1].rearrange("(n p) i -> p n i", p=128))
            def _qk_chunk(ring, half, cc):
                ring.dma_start(
                    out=qhalves[half][:, :, 512 * cc:512 * (cc + 1)],
                    in_=qk_d[256 * half:256 * (half + 1),
                             512 * cc:512 * (cc + 1)].rearrange(
                        "(n p) i -> p n i", p=128))
            # tiny bias/mask constants first (they gate the first epilogue
            # and the first diagonal mask), then q weights + q data + k
            # weights land in parallel on the three rings.  The ACT (scalar)
            # sequencer is NEVER used for DMA triggers: each trigger costs
            # ~667ns of sequencer time and delays the exp stream.
            gb_sb = const.tile([128, 8], F32, name="gb_sb")
            triu = const.tile([128, 128], BF16, name="triu")
            # round 1: everything the first Q/K projection + QK chunk needs
            # (ct=0 weight columns, q/k inputs S 0:512), as small pieces
            # racing down all three rings in parallel
            _wt_chunk(nc.scalar, 0, 0, 128)     # wq kc0 ct0
            _qk_piece(nc.gpsimd, 0, 0, 0, 512)  # xq kc0
            _qk_piece(nc.sync, 0, 1, 0, 512)    # xq kc1
            _wt_chunk(nc.scalar, 1, 0, 128)     # wq kc1 ct0
            _wt_chunk(nc.gpsimd, 2, 0, 128)     # wk kc0 ct0
            _wt_chunk(nc.sync, 3, 0, 128)       # wk kc1 ct0
            _qk_piece(nc.scalar, 1, 0, 0, 512)  # xk kc0
            _qk_piece(nc.gpsimd, 1, 1, 0, 512)  # xk kc1
            nc.scalar.dma_start(out=gb_sb,
                                in_=gb_d.rearrange("n (c p) -> p (n c)", p=128))
            nc.sync.dma_start(out=triu, in_=msk_d)
            # round 2: the rest, still roughly in first-need order
            _wt_chunk(nc.sync, 0, 128, 512)
            _wt_chunk(nc.scalar, 1, 128, 512)
            _wt_chunk(nc.gpsimd, 2, 128, 512)
            _wt_chunk(nc.sync, 3, 128, 512)
            _qk_piece(nc.scalar, 0, 0, 512, 1024)
            _qk_piece(nc.gpsimd, 0, 1, 512, 1024)
            _qk_piece(nc.sync, 1, 0, 512, 1024)
            _qk_piece(nc.scalar, 1, 1, 512, 1024)
            _wt_chunk(nc.gpsimd, 4)
            _wt_chunk(nc.sync, 5)
            wT = [[wt_sb[:, 2 * t + kc, :] for kc in range(2)] for t in range(3)]
            qT = [qhalves[0][:, 0, :], qhalves[0][:, 1, :]]
            kTt = [qhalves[1][:, 0, :], qhalves[1][:, 1, :]]
            bq_sb = gb_sb[:, 0:4]
            bk_sb = gb_sb[:, 4:8]
            bvb = const.tile([128, C], F32, name="bvb")
            nc.gpsimd.dma_start(
                out=bvb,
                in_=bass.AP(tensor=bv_d.tensor, offset=bv_d.offset,
                            ap=[[0, 128]] + list(bv_d.ap)),
            )

            QT = [persist.tile([128, S], BF16, name=f"QT{ct}") for ct in range(4)]
            KT = [persist.tile([128, S], BF16, name=f"KT{ct}") for ct in range(4)]
            VP = [persist.tile([128, NH * 65], BF16, name=f"VP{st}")
                  for st in range(8)]
            OUT = [persist.tile([128, C], F32, name=f"OUT{i}") for i in range(8)]

            with (
                tc.tile_pool(name="psW", bufs=1, space="PSUM") as psW,
                tc.tile_pool(name="psL", bufs=1, space="PSUM") as psL,
                tc.tile_pool(name="psPV", bufs=2, space="PSUM") as psPV,
                tc.tile_pool(name="epool", bufs=2) as epool,
            ):
                # ---- HAM warm-up: dummy matmuls bridge the input-DMA wait so
                # the PE clock gate reaches 8/8 by the first real matmul.
                for i in range(12):
                    wp = psW.tile([128, 512], F32, tag="pp", bufs=2,
                                  name=f"warm{i}")
                    nc.tensor.matmul(wp[:, 0:256], lhsT=wrm[:, 0:128],
                                     rhs=wrm[:, 0:256], start=True, stop=True)
                gchunk = [0]   # global chunk counter -> psL ping-pong parity

                def emit_proj_group(ct, g):
                    # g in 0..3 -> (q/k, n-half); scale pre-folded on host
                    dst, wpair, src, b_sb, pnm = (
                        (QT, wT[0], qT, bq_sb, "q"),
                        (KT, wT[1], kTt, bk_sb, "k"),
                    )[g // 2]
                    n = g % 2
                    pp = psW.tile([128, 512], F32, tag="pp", bufs=2,
                                  name=f"pp{pnm}{ct}_{n}")
                    for kc in range(2):
                        nc.tensor.matmul(
                            pp,
                            lhsT=wpair[kc][:, 128 * ct:128 * (ct + 1)],
                            rhs=src[kc][:, 512 * n:512 * (n + 1)],
                            start=(kc == 0), stop=(kc == 1),
                        )
                    # K carries no bias: logits (q+bq).(k+bk) differ from
                    # (q+bq).k only by a per-query-row constant q.bk, which
                    # cancels exactly in the softmax.  So K evicts as a plain
                    # Copy on the ACT engine (Copy is not table-based, so it
                    # never evicts the Exp table), offloading the DVE.
                    is_q = g // 2 == 0
                    if ct == 0 and g == 2:
                        # first QK matmul only needs KT[0][:, 0:128]
                        for c0, c1 in ((0, 128), (128, 512)):
                            nc.scalar.copy(dst[ct][:, c0:c1], pp[:, c0:c1])
                    elif is_q:
                        nc.vector.tensor_scalar_add(
                            out=dst[ct][:, 512 * n:512 * (n + 1)],
                            in0=pp,
                            scalar1=b_sb[:, ct:ct + 1],
                        )
                    else:
                        nc.scalar.copy(
                            dst[ct][:, 512 * n:512 * (n + 1)], pp)

                def emit_v(st):
                    vp = VP[st]
                    ppv = psW.tile([128, 512], F32, tag="pp", bufs=2,
                                   name=f"ppv{st}")
                    for kc in range(2):
                        nc.tensor.matmul(
                            ppv,
                            lhsT=kTt[kc][:, 128 * st:128 * (st + 1)],
                            rhs=wT[2][kc],
                            start=(kc == 0), stop=(kc == 1),
                        )
                    vp3 = vp.rearrange("p (h c) -> p h c", c=65)
                    nc.gpsimd.memset(vp3[:, :, 64:65], 1.0)
                    nc.vector.tensor_add(
                        vp3[:, :, 0:64],
                        ppv.rearrange("p (h c) -> p h c", c=64),
                        bvb.rearrange("p (h c) -> p h c", c=64),
                    )

                def emit_L(a2, c, segs, esegs):
                    W = sum(w for _, _, w in segs)
                    e = epool.tile([128, 2 * W], BF16, tag=f"e_{c}",
                                   name=f"e_{a2}_{c}")
                    off = 0
                    for (j, q0, w) in segs:
                        esegs.setdefault(j, []).append((e, W, off, q0, w))
                        off += w
                    # full [128, 2, 512] so the head planes stay bank-aligned;
                    # parity from a global counter so phase boundaries keep
                    # ping-ponging instead of colliding on the same buffer
                    gchunk[0] += 1
                    lt = psL.tile([128, 2, 512], F32, tag=f"lt{gchunk[0] % 2}",
                                  name=f"lt_{a2}_{c}")
                    for hi in range(2):
                        p0 = 64 * hi
                        off = 0
                        for (j, q0, w) in segs:
                            nc.tensor.matmul(
                                lt[:, hi, off:off + w],
                                lhsT=KT[a2][p0:p0 + 64, 128 * j:128 * j + 128],
                                rhs=QT[a2][p0:p0 + 64, q0:q0 + w],
                                start=True, stop=True,
                            )
                            off += w
                    if c in SCHRAUD_AT:
                        # one-op DVE Schraudolph fast-exp: int16(l*C0 + C1)
                        # written into the e tile viewed as int16 IS bf16
                        # ~exp(l/8).  Frees ACT time at no extra passes.
                        nc.vector.tensor_scalar(
                            out=e.rearrange("p (h c) -> p h c",
                                            h=2).bitcast(mybir.dt.int16),
                            in0=lt[:, :, 0:W],
                            scalar1=_SCH_C0, scalar2=_SCH_C1,
                            op0=ALU.mult, op1=ALU.add)
                    else:
                        # one exp covering both heads
                        nc.scalar.activation(
                            out=e.rearrange("p (h c) -> p h c", h=2),
                            in_=lt[:, :, 0:W], func=AF.Exp, scale=0.125)
                    # one strided-AP strict-upper mask multiply covering the
                    # diagonal tiles of both heads (segs starting on-diagonal)
                    dsegs = [off_ for (j, q0, w), off_ in zip(
                        segs, np.cumsum([0] + [w for _, _, w in segs[:-1]]))
                        if q0 == 128 * j]
                    if dsegs:
                        dims = [list(e.ap[0]), [W, 2]]
                        tdims = [list(triu.ap[0]), [0, 2]]
                        if len(dsegs) == 2:
                            dims.append([dsegs[1] - dsegs[0], 2])
                            tdims.append([0, 2])
                        dims.append([1, 128])
                        tdims.append([1, 128])
                        ev = bass.AP(tensor=e.tensor,
                                     offset=e.offset + int(dsegs[0]), ap=dims)
                        tv = bass.AP(tensor=triu.tensor, offset=triu.offset,
                                     ap=tdims)
                        nc.vector.tensor_mul(ev, ev, tv)

                def _pv_acc(po, a2, i, esegs, jj0, jj1, start, stop):
                    # accumulate strips jj0..jj1 of PV(i) for the head pair;
                    # start/stop open/close the psum accumulation group so a
                    # chain can be split across chunks (deferred tail terms).
                    # PSUM zero-regions are bank-sized: exactly ONE start=True
                    # is issued per bank (the lazy zero covers the second
                    # head's bytes on their first write) so a later start can
                    # never re-mark already-accumulated bytes pending-zero.
                    for hi in range(2):
                        hh = 2 * a2 + hi
                        for jj in range(jj0, jj1 + 1):
                            for (et, W, off, q0, w) in esegs[jj]:
                                if q0 <= 128 * i < q0 + w:
                                    base = hi * W + off + 128 * i - q0
                                    break
                            nc.tensor.matmul(
                                po[:, 65 * hi:65 * hi + 65],
                                lhsT=et[:, base:base + 128],
                                rhs=VP[jj][:, 65 * hh:65 * hh + 65],
                                start=(start and hi == 0 and jj == jj0),
                                stop=(stop and hi == 1 and jj == jj1),
                                skip_group_check=True,
                            )

                def norm_po(a2, i, po):
                    r = smalls.tile([128, 2], F32, tag="r", name=f"r{a2}_{i}")
                    nc.vector.reciprocal_approx_fast(
                        r, po.rearrange("p (g x) -> p g x", g=2)[:, :, 64:65])
                    # one strided-AP normalize covering both heads
                    r_v = bass.AP(tensor=r.tensor, offset=r.offset,
                                  ap=[list(r.ap[0]), [1, 2], [0, 64]])
                    nc.vector.tensor_mul(
                        OUT[i][:, 128 * a2:128 * (a2 + 1)].rearrange(
                            "p (h c) -> p h c", c=64),
                        po.rearrange("p (h c) -> p h c", c=65)[:, :, 0:64],
                        r_v,
                    )

                def emit_PV(a2, i, esegs):
                    # both heads accumulate into one 1-bank psum tile
                    po = psPV.tile([128, 130], F32, tag="po",
                                   name=f"po_{a2}_{i}")
                    _pv_acc(po, a2, i, esegs, 0, i, start=True, stop=True)
                    norm_po(a2, i, po)

                def emit_PV_part(a2, i, esegs, jj0, jj1, tag):
                    # a complete closed accumulation group over jj0..jj1
                    pool = psW if tag == "pp" else psPV
                    po = pool.tile([128, 130], F32, tag=tag, bufs=2,
                                   name=f"pop_{a2}_{i}_{jj0}")
                    for hi in range(2):
                        hh = 2 * a2 + hi
                        for jj in range(jj0, jj1 + 1):
                            for (et, W, off, q0, w) in esegs[jj]:
                                if q0 <= 128 * i < q0 + w:
                                    base = hi * W + off + 128 * i - q0
                                    break
                            nc.tensor.matmul(
                                po[:, 65 * hi:65 * hi + 65],
                                lhsT=et[:, base:base + 128],
                                rhs=VP[jj][:, 65 * hh:65 * hh + 65],
                                start=(jj == jj0), stop=(jj == jj1),
                            )
                    return po

                def finish_tail(i, pa_sbuf, pb):
                    # merge the two partials on DVE, then normalize
                    m = smalls.tile([128, 130], F32, tag="m", name=f"m{i}")
                    nc.vector.tensor_add(m, pa_sbuf, pb)
                    r = smalls.tile([128, 2], F32, tag="r", name=f"rt{i}")
                    nc.vector.reciprocal(
                        r, m.rearrange("p (g x) -> p g x", g=2)[:, :, 64:65])
                    r_v = bass.AP(tensor=r.tensor, offset=r.offset,
                                  ap=[list(r.ap[0]), [1, 2], [0, 64]])
                    nc.vector.tensor_mul(
                        OUT[i][:, 384:512].rearrange("p (h c) -> p h c", c=64),
                        m.rearrange("p (h c) -> p h c", c=65)[:, :, 0:64],
                        r_v,
                    )

                # outputs all ride the SP HWDGE ring: SP's sequencer is idle
                # and HWDGE avoids the Q7 SWDGE drain that stretched the tail
                out_rings = [nc.sync, nc.sync]

                def post_pv(a2, i):
                    # ship each half of OUT[i] as soon as its last head-pair
                    # phase has normalized into it (query row 0 attends to
                    # nothing: reference zeroes it first)
                    if a2 == 1:
                        if i == 0:
                            nc.vector.memset(OUT[0][0:1, 0:256], 0.0)
                        out_rings[i % 2].dma_start(
                            out=o_d[128 * i:128 * (i + 1), 0:256],
                            in_=OUT[i][:, 0:256])
                    elif a2 == 3:
                        if i == 0:
                            nc.vector.memset(OUT[0][0:1, 256:512], 0.0)
                        out_rings[i % 2].dma_start(
                            out=o_d[128 * i:128 * (i + 1), 256:512],
                            in_=OUT[i][:, 256:512])

                emit_proj_group(0, 0)   # Q-n0
                emit_proj_group(0, 2)   # K-n0: chunks 0-3 need nothing else
                PROJ_AT = {2: 0, 4: 1, 5: 2, 7: 3}
                carry = []
                po6 = po7 = None
                for a2 in range(4):
                    chunks = CHUNKS if a2 < 3 else CHUNKS3
                    esegs = {}
                    for c in range(len(chunks)):
                        if a2 == 0 and c == 3:
                            emit_proj_group(0, 1)   # Q-n1 (data lands late)
                        if a2 == 0 and c == 5:
                            emit_proj_group(0, 3)   # K-n1
                        if a2 < 3 and c in PROJ_AT:
                            emit_proj_group(a2 + 1, PROJ_AT[c])
                        emit_L(a2, c, chunks[c], esegs)
                        if a2 == 0:
                            for i in V_AT.get(c, []):
                                emit_v(i)
                        if c < len(carry):
                            pa, pi, pes = carry[c]
                            emit_PV(pa, pi, pes)
                            post_pv(pa, pi)
                        # last phase: split the two tail PV chains so only
                        # their final diagonal terms trail the last exps.
                        # Deferred closes are emitted BEFORE any new psPV
                        # allocation at the same chunk so pool buffer reuse
                        # never overtakes an open chain.
                        if a2 == 3 and c == 9:
                            _pv_acc(po6, 3, 6, esegs, 6, 6,
                                    start=False, stop=True)
                            norm_po(3, 6, po6)
                            post_pv(3, 6)
                            _pv_acc(po7, 3, 7, esegs, 6, 6,
                                    start=False, stop=False)
                        for i in PV_AT.get(c, []):
                            emit_PV(a2, i, esegs)
                            post_pv(a2, i)
                        if a2 == 3 and c == 8:
                            po6 = psPV.tile([128, 130], F32, tag="po",
                                            name="po_d6")
                            _pv_acc(po6, 3, 6, esegs, 0, 5,
                                    start=True, stop=False)
                            po7 = psPV.tile([128, 130], F32, tag="po",
                                            name="po_d7")
                            _pv_acc(po7, 3, 7, esegs, 0, 5,
                                    start=True, stop=False)
                        if a2 == 3 and c == 10:
                            _pv_acc(po7, 3, 7, esegs, 7, 7,
                                    start=False, stop=True)
                            norm_po(3, 7, po7)
                            post_pv(3, 7)
                    carry = [(a2, 6, esegs), (a2, 7, esegs)]
    nc.compile()
    return nc


_CACHE = {}


def _get_module():
    if "nc" not in _CACHE:
        _CACHE["nc"] = _build_module()
    return _CACHE["nc"]


def _in_maps(inputs):
    import ml_dtypes

    q = np.asarray(inputs["query"], dtype=np.float32)
    k = np.asarray(inputs["key"], dtype=np.float32)
    B = q.shape[0]
    assert B == N_CORES
    # fold weight-norm scale g/||v_row|| into the weights; pre-transpose
    wts = []
    for nm in ("q", "k", "v"):
        v = np.asarray(inputs[f"v{nm}"], np.float64)
        g = np.asarray(inputs[f"g{nm}"], np.float64)
        w = g[:, None] * v / np.linalg.norm(v, axis=1, keepdims=True)
        wts.append(w.T)                               # [CIN, C]
    wt = np.ascontiguousarray(
        np.concatenate(wts, axis=0).astype(ml_dtypes.bfloat16))
    gb = np.ascontiguousarray(np.stack(
        [np.asarray(inputs["bq"], np.float32),
         np.asarray(inputs["bk"], np.float32)]))
    bv = np.ascontiguousarray(np.asarray(inputs["bv"], np.float32))
    msk = np.ascontiguousarray(
        np.triu(np.ones((128, 128), np.float32), k=1).astype(ml_dtypes.bfloat16))
    shared = {"wt": wt, "gb": gb, "bv": bv, "msk": msk}
    maps = []
    for b in range(B):
        m = dict(shared)
        m["qk"] = np.ascontiguousarray(np.concatenate(
            [q[b].reshape(CIN, S), k[b].reshape(CIN, S)], axis=0
        ).astype(ml_dtypes.bfloat16))
        maps.append(m)
    return maps


def _gather(results):
    outs = []
    for b in range(N_CORES):
        o = np.asarray(results[b]["o"], np.float32)   # [S, C]
        outs.append(np.ascontiguousarray(o.T).reshape(C, HW, HW))
    return np.stack(outs).astype(np.float32)      # [B, C, H, W]


def run(inputs, **kw):
    """Run on hardware; returns (full_output, BassKernelResults)."""
    nc = _get_module()
    res = run_bass_kernel_spmd(nc, _in_maps(inputs), list(range(N_CORES)), **kw)
    return _gather(res.results), res


def kernel(**inputs):
    out, _ = run(inputs)
    return out

